# revision 1
# baseline (speedup 1.0000x reference)
import sys
import os

sys.path.insert(0, "/opt/trn_rl_repo")

import numpy as np
import ml_dtypes

import concourse.bacc as bacc
import concourse.bass as bass
import concourse.tile as tile
from concourse import mybir
from concourse.bass_utils import run_bass_kernel_spmd

f32 = mybir.dt.float32
f32r = mybir.dt.float32r
bf16 = mybir.dt.bfloat16
u32 = mybir.dt.uint32
i32 = mybir.dt.int32

# problem geometry (hardcoded; kernel.py must be self-contained)
H = W = 64
C = 3
K = 32
PAD = 10
OH = H + 2 * PAD - K + 1          # 53
L = OH * OH                       # 2809
MT = 22                           # m-tiles of 128 rows: 22*128 = 2816 >= L
LP = MT * 128
D = C * K * K                     # 3072
DSTEP = D // 128                  # 24
N_MEM = 20000
NCORES = 8
NK = N_MEM // NCORES              # 2500 keys per core
NSUB = 5                          # column tiles of 512
SUBW = 512
NKP = NSUB * SUBW                 # 2560 (padded)

MODE = os.environ.get("KMODE", "bf16x3")   # "bf16x3" or "f32r"
TRACE = False
LAST_EXEC_NS = None
LAST_RESULTS = None

_NC_CACHE = {}


def _build(mode):
    nc = bacc.Bacc("TRN2", target_bir_lowering=False, debug=False,
                   num_devices=NCORES)

    if mode == "bf16x3":
        qh_d = nc.dram_tensor("qh", [MT, 128, DSTEP, 128], bf16,
                              kind="ExternalInput")
        ql_d = nc.dram_tensor("ql", [MT, 128, DSTEP, 128], bf16,
                              kind="ExternalInput")
        kh_d = nc.dram_tensor("kh", [NSUB, 128, DSTEP, SUBW], bf16,
                              kind="ExternalInput")
        kl_d = nc.dram_tensor("kl", [NSUB, 128, DSTEP, SUBW], bf16,
                              kind="ExternalInput")
        groups = [[0, 1, 2], [3, 4]]
    else:
        qf_d = nc.dram_tensor("qf", [MT, 128, DSTEP, 128], f32r,
                              kind="ExternalInput")
        kf_d = nc.dram_tensor("kf", [NSUB, 128, DSTEP, SUBW], f32r,
                              kind="ExternalInput")
        groups = [[0, 1], [2, 3], [4]]

    bias_d = nc.dram_tensor("bias", [128, NKP], f32, kind="ExternalInput")
    bases_d = nc.dram_tensor("bases", [128, NSUB], f32, kind="ExternalInput")
    vals_d = nc.dram_tensor("vals", [(NK + 1) * K, C, K], f32,
                            kind="ExternalInput")
    ones_d = nc.dram_tensor("ones", [1, 64], f32, kind="ExternalInput")

    out_d = nc.dram_tensor("out", [64, C, 64], f32, kind="ExternalOutput")
    gi_d = nc.dram_tensor("gi", [128, MT], f32, kind="ExternalOutput")

    with tile.TileContext(nc) as tc:
        with (
            tc.tile_pool(name="keys", bufs=1) as kpool,
            tc.tile_pool(name="qp", bufs=2) as qpool,
            tc.tile_pool(name="work", bufs=1) as wpool,
            tc.tile_pool(name="sm", bufs=2) as mpool,
            tc.tile_pool(name="psum", bufs=2, space=bass.MemorySpace.PSUM) as ppool,
            tc.tile_pool(name="stg", bufs=6) as spool,
            tc.tile_pool(name="dram", bufs=1, space="DRAM") as dpool,
        ):
            tbias = wpool.tile([128, NKP], f32)
            nc.sync.dma_start(tbias[:], bias_d[:])
            tbases = wpool.tile([128, NSUB], f32)
            nc.sync.dma_start(tbases[:], bases_d[:])
            tones = wpool.tile([1, 64], f32)
            nc.sync.dma_start(tones[:], ones_d[:])

            best = wpool.tile([128, MT], f32)
            bix = wpool.tile([128, MT], f32)

            # ---------------- scan: scores + per-core argmax ----------------
            for gidx, group in enumerate(groups):
                g0, glen = group[0], len(group)
                if mode == "bf16x3":
                    kht = kpool.tile([128, glen, DSTEP, SUBW], bf16)
                    klt = kpool.tile([128, glen, DSTEP, SUBW], bf16)
                    nc.sync.dma_start(
                        kht[:], kh_d[g0:g0 + glen].transpose([1, 0, 2, 3])[:])
                    nc.sync.dma_start(
                        klt[:], kl_d[g0:g0 + glen].transpose([1, 0, 2, 3])[:])
                else:
                    kft = kpool.tile([128, glen, DSTEP, SUBW], f32r)
                    nc.sync.dma_start(
                        kft[:], kf_d[g0:g0 + glen].transpose([1, 0, 2, 3])[:])

                for m in range(MT):
                    if mode == "bf16x3":
                        qht = qpool.tile([128, DSTEP, 128], bf16)
                        qlt = qpool.tile([128, DSTEP, 128], bf16)
                        nc.sync.dma_start(qht[:], qh_d[m])
                        nc.sync.dma_start(qlt[:], ql_d[m])
                    else:
                        qft = qpool.tile([128, DSTEP, 128], f32r)
                        nc.sync.dma_start(qft[:], qf_d[m])

                    for si in range(glen):
                        s = g0 + si
                        acc = ppool.tile([128, SUBW], f32)
                        if mode == "bf16x3":
                            passes = [(qht, kht), (qht, klt), (qlt, kht)]
                        else:
                            passes = [(qft, kft)]
                        nmm = DSTEP * len(passes)
                        i = 0
                        for d in range(DSTEP):
                            for (lt, rt) in passes:
                                nc.tensor.matmul(acc[:], lt[:, d, :],
                                                 rt[:, si, d, :],
                                                 start=(i == 0),
                                                 stop=(i == nmm - 1))
                                i += 1

                        sc = mpool.tile([128, SUBW], f32)
                        nc.vector.scalar_tensor_tensor(
                            sc[:], acc[:], 1.0,
                            tbias[:, s * SUBW:(s + 1) * SUBW],
                            op0=mybir.AluOpType.mult,
                            op1=mybir.AluOpType.add)
                        mxv = mpool.tile([128, 8], f32)
                        mxi = mpool.tile([128, 8], u32)
                        nc.vector.max_with_indices(mxv[:], mxi[:], sc[:])
                        nixf = mpool.tile([128, 1], f32)
                        nc.vector.tensor_copy(nixf[:], mxi[:, 0:1])
                        nix2 = mpool.tile([128, 1], f32)
                        nc.vector.tensor_scalar_add(nix2[:], nixf[:],
                                                    tbases[:, s:s + 1])
                        if gidx == 0 and si == 0:
                            nc.vector.tensor_copy(best[:, m:m + 1],
                                                  mxv[:, 0:1])
                            nc.vector.tensor_copy(bix[:, m:m + 1], nix2[:])
                        else:
                            gt = mpool.tile([128, 1], u32)
                            nc.vector.scalar_tensor_tensor(
                                gt[:], mxv[:, 0:1], 1.0, best[:, m:m + 1],
                                op0=mybir.AluOpType.mult,
                                op1=mybir.AluOpType.is_gt)
                            nc.vector.copy_predicated(best[:, m:m + 1],
                                                      gt[:], mxv[:, 0:1])
                            nc.vector.copy_predicated(bix[:, m:m + 1],
                                                      gt[:], nix2[:])

            # ------------- global argmin via AllReduce(max)+(min) -----------
            cc1 = dpool.tile([128, MT], f32)
            cc2 = dpool.tile([128, MT], f32)
            nc.gpsimd.dma_start(cc1[:], best[:])
            nc.gpsimd.collective_compute(
                "AllReduce", mybir.AluOpType.max,
                replica_groups=[list(range(NCORES))],
                ins=[cc1.opt()], outs=[cc2.opt()])
            gbest = wpool.tile([128, MT], f32)
            nc.gpsimd.dma_start(gbest[:], cc2[:])

            ge = wpool.tile([128, MT], f32)
            nc.vector.scalar_tensor_tensor(
                ge[:], best[:], 1.0, gbest[:],
                op0=mybir.AluOpType.mult, op1=mybir.AluOpType.is_ge)
            t1 = wpool.tile([128, MT], f32)
            nc.vector.tensor_scalar_add(t1[:], bix[:], -1.0e6)
            t2 = wpool.tile([128, MT], f32)
            nc.vector.scalar_tensor_tensor(
                t2[:], ge[:], 1.0, t1[:],
                op0=mybir.AluOpType.mult, op1=mybir.AluOpType.mult)
            cand = wpool.tile([128, MT], f32)
            nc.vector.tensor_scalar_add(cand[:], t2[:], 1.0e6)

            cc3 = dpool.tile([128, MT], f32)
            cc4 = dpool.tile([128, MT], f32)
            nc.gpsimd.dma_start(cc3[:], cand[:])
            nc.gpsimd.collective_compute(
                "AllReduce", mybir.AluOpType.min,
                replica_groups=[list(range(NCORES))],
                ins=[cc3.opt()], outs=[cc4.opt()])
            gif = wpool.tile([128, MT], f32)
            nc.gpsimd.dma_start(gif[:], cc4[:])
            nc.sync.dma_start(gi_d[:], gif[:])

            # local row index: owned -> gi - c*2500, else dummy row NK
            li = wpool.tile([128, MT], f32)
            nc.vector.tensor_scalar(li[:], gif[:], tbases[:, 0:1], None,
                                    op0=mybir.AluOpType.subtract)
            o1 = wpool.tile([128, MT], f32)
            nc.vector.tensor_scalar(o1[:], li[:], 0.0, None,
                                    op0=mybir.AluOpType.is_ge)
            o2 = wpool.tile([128, MT], f32)
            nc.vector.tensor_scalar(o2[:], li[:], float(NK), None,
                                    op0=mybir.AluOpType.is_lt)
            own = wpool.tile([128, MT], f32)
            nc.vector.scalar_tensor_tensor(
                own[:], o1[:], 1.0, o2[:],
                op0=mybir.AluOpType.mult, op1=mybir.AluOpType.mult)
            d1 = wpool.tile([128, MT], f32)
            nc.vector.tensor_scalar_add(d1[:], li[:], -float(NK))
            d2 = wpool.tile([128, MT], f32)
            nc.vector.scalar_tensor_tensor(
                d2[:], own[:], 1.0, d1[:],
                op0=mybir.AluOpType.mult, op1=mybir.AluOpType.mult)
            lc = wpool.tile([128, MT], f32)
            nc.vector.tensor_scalar_add(lc[:], d2[:], float(NK))
            lc32 = wpool.tile([128, MT], f32)
            nc.vector.tensor_scalar_mul(lc32[:], lc[:], float(K))
            gi32s = wpool.tile([128, MT], i32)
            nc.vector.tensor_copy(gi32s[:], lc32[:])

            # relayout [128, MT] -> [1, LP] (partition 0) via DRAM bounce:
            # flat index pid = m*128 + p must read gi32s[p, m]
            gidr = dpool.tile([128, MT], i32)
            nc.sync.dma_start(gidr[:], gi32s[:])
            gi32 = wpool.tile([1, LP], i32)
            nc.sync.dma_start(gi32[:], gidr.transpose([1, 0])[:])

            # --------------------- gather + fold ---------------------------
            # band accumulator: partition r holds the in-flight canvas row
            # with row_idx % K == r (each 32-row band covers every residue
            # exactly once). Gather DMA rotates patch rows into residue
            # order so compute APs always start at partition 0.
            acc = wpool.tile([K, C, H + 2 * PAD], f32)
            nc.vector.memset(acc[:], 0.0)
            zrow = wpool.tile([1, C, H + 2 * PAD], f32)
            nc.vector.memset(zrow[:], 0.0)
            cc5 = dpool.tile([H + 2 * PAD, C, H + 2 * PAD], f32)
            cc6 = dpool.tile([H + 2 * PAD, C, H + 2 * PAD], f32)

            for oh in range(OH):
                s = oh % K
                for ow in range(OH):
                    pid = oh * OH + ow
                    iv = nc.values_load(gi32[0:1, pid:pid + 1],
                                        engines=(mybir.EngineType.SP,),
                                        min_val=0, max_val=NK * K,
                                        skip_runtime_bounds_check=True)
                    stg = spool.tile([K, C, K], f32)
                    nc.sync.dma_start(stg[s:K, :, :],
                                      vals_d[bass.ds(iv, K - s), :, :])
                    if s:
                        nc.sync.dma_start(
                            stg[0:s, :, :],
                            vals_d[bass.ds(iv + (K - s), s), :, :])
                    nc.vector.scalar_tensor_tensor(
                        acc[:, :, ow:ow + K], stg[:], 1.0,
                        acc[:, :, ow:ow + K],
                        op0=mybir.AluOpType.mult, op1=mybir.AluOpType.add)
                # canvas row oh is complete -> flush, then reset that
                # partition for row oh+K
                rf = oh % K
                nc.sync.dma_start(cc5[oh:oh + 1, :, :], acc[rf:rf + 1, :, :])
                if oh < OH - 1:
                    nc.sync.dma_start(acc[rf:rf + 1, :, :], zrow[:])
            for r in range(OH, H + 2 * PAD):
                nc.sync.dma_start(cc5[r:r + 1, :, :],
                                  acc[r % K:r % K + 1, :, :])

            # sum partial canvases across cores
            nc.gpsimd.collective_compute(
                "AllReduce", mybir.AluOpType.add,
                replica_groups=[list(range(NCORES))],
                ins=[cc5.opt()], outs=[cc6.opt()])

            # --------------------- normalize -------------------------------
            crop_s = wpool.tile([H, C, W], f32)
            nc.sync.dma_start(crop_s[:], cc6[PAD:PAD + H, :, PAD:PAD + W])
            crop = crop_s[:]
            rowmax = wpool.tile([H, 1], f32)
            nc.vector.tensor_reduce(rowmax[:], crop,
                                    mybir.AxisListType.XY,
                                    mybir.AluOpType.max)
            drmax = dpool.tile([H, 1], f32)
            nc.sync.dma_start(drmax[:], rowmax[:])
            rmT = wpool.tile([1, H], f32)
            nc.sync.dma_start(rmT[:], drmax.transpose([1, 0])[:])
            gmax = wpool.tile([1, 1], f32)
            nc.vector.tensor_reduce(gmax[:], rmT[:],
                                    mybir.AxisListType.X,
                                    mybir.AluOpType.max)
            pb = ppool.tile([H, 1], f32)
            nc.tensor.matmul(pb[:], tones[:], gmax[:], start=True, stop=True)
            gmb = wpool.tile([H, 1], f32)
            nc.vector.tensor_copy(gmb[:], pb[:])
            rcp = wpool.tile([H, 1], f32)
            nc.vector.reciprocal(rcp[:], gmb[:])
            outn = wpool.tile([H, C, W], f32)
            nc.vector.tensor_scalar(outn[:], crop, rcp[:, 0:1], None,
                                    op0=mybir.AluOpType.mult)
            nc.sync.dma_start(out_d[:], outn[:])

    nc.compile()
    return nc


def _get_nc(mode):
    if mode not in _NC_CACHE:
        _NC_CACHE[mode] = _build(mode)
    return _NC_CACHE[mode]


def _im2col(image):
    img = np.ascontiguousarray(image.transpose(2, 0, 1)).astype(np.float32)
    xp = np.pad(img, ((0, 0), (PAD, PAD), (PAD, PAD)))
    win = np.arange(OH)[:, None] + np.arange(K)[None, :]
    p = xp[:, win[:, None, :, None], win[None, :, None, :]]
    return p.transpose(1, 2, 0, 3, 4).reshape(L, D)


def _to6(x, rows, tiles, width):
    # (rows, D) -> (tiles, 128, DSTEP, width) lhsT/rhs layout
    return np.ascontiguousarray(
        x.T.reshape(DSTEP, 128, tiles, width).transpose(2, 1, 0, 3))


def _prepare_in_maps(image, mem_keys, mem_values, mode):
    q = _im2col(image)
    qpad = np.zeros((LP, D), dtype=np.float32)
    qpad[:L] = q

    shared = {}
    if mode == "bf16x3":
        qh = qpad.astype(ml_dtypes.bfloat16)
        ql = (qpad - qh.astype(np.float32)).astype(ml_dtypes.bfloat16)
        shared["qh"] = _to6(qh, LP, MT, 128)
        shared["ql"] = _to6(ql, LP, MT, 128)
    else:
        shared["qf"] = _to6(qpad, LP, MT, 128)
    shared["ones"] = np.ones((1, 64), dtype=np.float32)

    in_maps = []
    for c in range(NCORES):
        kc = mem_keys[c * NK:(c + 1) * NK]
        kcp = np.zeros((NKP, D), dtype=np.float32)
        kcp[:NK] = kc
        m = dict(shared)
        if mode == "bf16x3":
            kh = kcp.astype(ml_dtypes.bfloat16)
            kl = (kcp - kh.astype(np.float32)).astype(ml_dtypes.bfloat16)
            m["kh"] = _to6(kh, NKP, NSUB, SUBW)
            m["kl"] = _to6(kl, NKP, NSUB, SUBW)
        else:
            m["kf"] = _to6(kcp, NKP, NSUB, SUBW)

        n2 = (kc.astype(np.float64) ** 2).sum(axis=1)
        bias = np.full(NKP, -1.0e9, dtype=np.float32)
        bias[:NK] = (-0.5 * n2).astype(np.float32)
        m["bias"] = np.ascontiguousarray(
            np.broadcast_to(bias, (128, NKP))).astype(np.float32)

        bases = (c * NK + np.arange(NSUB, dtype=np.float32) * SUBW)
        m["bases"] = np.ascontiguousarray(
            np.broadcast_to(bases, (128, NSUB))).astype(np.float32)

        vc = mem_values[c * NK:(c + 1) * NK]
        v4 = np.zeros(((NK + 1), K, C, K), dtype=np.float32)
        v4[:NK] = vc.reshape(NK, C, K, K).transpose(0, 2, 1, 3)
        m["vals"] = v4.reshape((NK + 1) * K, C, K)

        in_maps.append(m)
    return in_maps


def kernel(**inputs):
    global LAST_EXEC_NS, LAST_RESULTS
    image = np.asarray(inputs["image"], dtype=np.float32)
    mem_keys = np.asarray(inputs["mem_keys"], dtype=np.float32)
    mem_values = np.asarray(inputs["mem_values"], dtype=np.float32)

    mode = MODE
    nc = _get_nc(mode)
    in_maps = _prepare_in_maps(image, mem_keys, mem_values, mode)

    r = run_bass_kernel_spmd(nc, in_maps, list(range(NCORES)), trace=TRACE)
    LAST_EXEC_NS = r.exec_time_ns
    LAST_RESULTS = r.results
    out = r.results[0]["out"]
    return np.ascontiguousarray(out.transpose(0, 2, 1)).astype(np.float32)


if __name__ == "__main__":
    rng = np.random.default_rng(0)
    ins = dict(
        image=rng.random((H, W, C), dtype=np.float32),
        mem_keys=rng.standard_normal((N_MEM, D), dtype=np.float32),
        mem_values=rng.standard_normal((N_MEM, D), dtype=np.float32),
    )
    o = kernel(**ins)
    print("out", o.shape, o.dtype, float(o.max()), float(o.min()))



# revision 25
# speedup vs baseline: 1.2034x; 1.2034x over previous
import sys
import os

sys.path.insert(0, "/opt/trn_rl_repo")

import numpy as np
import ml_dtypes

import concourse.bacc as bacc
import concourse.bass as bass
import concourse.tile as tile
from concourse import mybir
from concourse.bass import IndirectOffsetOnAxis
from concourse.bass_isa import ReduceOp
from concourse.bass_utils import run_bass_kernel_spmd
from concourse.masks import make_identity

f32 = mybir.dt.float32
bf16 = mybir.dt.bfloat16
u32 = mybir.dt.uint32
i32 = mybir.dt.int32

# problem geometry (hardcoded; kernel.py must be self-contained)
H = W = 64
C = 3
K = 32
PAD = 10
OH = H + 2 * PAD - K + 1          # 53
L = OH * OH                       # 2809
MT = 22                           # m-tiles of 128 rows: 22*128 = 2816 >= L
LP = MT * 128
D = C * K * K                     # 3072
DSTEP = D // 128                  # 24
N_MEM = 20000
NCORES = 8
NK = N_MEM // NCORES              # 2500 keys per core
NSUB = 5                          # column tiles of 512
SUBW = 512
NKP = NSUB * SUBW                 # 2560 (padded)
DE = D + 64                       # 3136 extended rescore row (q.k | bias)
NCAND = 3                         # exact-rescored candidates per patch
# fold geometry: values table split as [row*2 + kw//16, (c, kh, kw%16)]
KWG = 16                          # kw group width
NKWO = K // KWG                   # 2
VROWS = N_MEM * NKWO + 4          # + dummy zero rows
FSEG = C * K * KWG                # 1536
FG = 8                            # oh rows per fold group
NFG = (OH + FG - 1) // FG         # 7 groups
FW_PAD = LP + 8                   # flat winner array (oh-major)
FWT_PAD = 64 * OH + 8             # ow-major winner array + pad zone

MODE = "bf16_rescore"
DEBUG = False
TRACE = False
LAST_EXEC_NS = None
LAST_RESULTS = None

_NC_CACHE = {}

AL = mybir.AluOpType


def _build(mode):
    nc = bacc.Bacc("TRN2", target_bir_lowering=False, debug=False,
                   num_devices=NCORES)

    qh_d = nc.dram_tensor("qh", [MT, 128, DSTEP, 128], bf16,
                          kind="ExternalInput")
    kh_d = nc.dram_tensor("kh", [NSUB, 128, DSTEP, SUBW], bf16,
                          kind="ExternalInput")
    bias_d = nc.dram_tensor("bias", [128, NKP], f32, kind="ExternalInput")
    bases_d = nc.dram_tensor("bases", [128, NSUB], f32, kind="ExternalInput")
    cbase_d = nc.dram_tensor("cbase", [128, 1], f32, kind="ExternalInput")
    kext_d = nc.dram_tensor("kext", [NKP, DE], f32, kind="ExternalInput")
    qext_d = nc.dram_tensor("qext", [MT, 128, DE], f32, kind="ExternalInput")
    vals_d = nc.dram_tensor("vals", [VROWS, FSEG], bf16, kind="ExternalInput")
    bcol_d = nc.dram_tensor("bcol", [KWG, 128, 128], bf16,
                            kind="ExternalInput")
    pt2_d = nc.dram_tensor("pt2", [128, 1], f32, kind="ExternalInput")
    pkwo_d = nc.dram_tensor("pkwo", [128, 1], f32, kind="ExternalInput")

    out_d = nc.dram_tensor("out", [64, 64, C], f32, kind="ExternalOutput")
    gi_d = nc.dram_tensor("gi", [128, MT], f32, kind="ExternalOutput")
    fwin = nc.dram_tensor("fwin", [1, FW_PAD], f32, kind="Internal")
    fwinT = nc.dram_tensor("fwinT", [1, FWT_PAD], f32, kind="Internal")
    if DEBUG:
        dbg_idx = nc.dram_tensor("dbg_idx", [128, FG], f32,
                                 kind="ExternalOutput")
        dbg_gv = nc.dram_tensor("dbg_gv", [128, FSEG], f32,
                                kind="ExternalOutput")
        dbg_cv = nc.dram_tensor("dbg_cv", [128, H + 2 * PAD, C], f32,
                                kind="ExternalOutput")
        dbg_fw = nc.dram_tensor("dbg_fw", [128, MT], f32,
                                kind="ExternalOutput")

    with tile.TileContext(nc) as tc:
        with (
            tc.tile_pool(name="keys", bufs=1) as kpool,
            tc.tile_pool(name="qp", bufs=2) as qpool,
            tc.tile_pool(name="qe", bufs=1) as qepool,
            tc.tile_pool(name="gp", bufs=2) as gpool,
            tc.tile_pool(name="work", bufs=1) as wpool,
            tc.tile_pool(name="sm", bufs=2) as mpool,
            tc.tile_pool(name="stg", bufs=2) as spool,
            tc.tile_pool(name="dram", bufs=1, space="DRAM") as dpool,
        ):
            # ---- resident staging ----
            kht = kpool.tile([128, NSUB, DSTEP, SUBW], bf16, tag="big")
            nc.sync.dma_start(kht[:], kh_d.transpose([1, 0, 2, 3])[:])
            tbias = wpool.tile([128, NKP], f32)
            nc.sync.dma_start(tbias[:], bias_d[:])
            tbases = wpool.tile([128, NSUB], f32)
            nc.sync.dma_start(tbases[:], bases_d[:])
            tcbase = wpool.tile([128, 1], f32)
            nc.sync.dma_start(tcbase[:], cbase_d[:])
            tpkwo = wpool.tile([128, 1], f32)
            nc.sync.dma_start(tpkwo[:], pkwo_d[:])
            tpt2 = wpool.tile([128, 1], f32)
            nc.sync.dma_start(tpt2[:], pt2_d[:])
            tbcol = wpool.tile([128, KWG, 128], bf16)
            nc.sync.dma_start(tbcol[:], bcol_d.transpose([1, 0, 2])[:])
            ident = wpool.tile([128, 128], f32)
            make_identity(nc, ident[:])

            best = wpool.tile([128, MT], f32)
            bix = wpool.tile([128, MT], f32)
            dummy = wpool.tile([128, 1], f32)

            # =================== scan: bf16 1-pass + exact top-3 rescore
            with tc.tile_pool(name="pscan", bufs=1,
                              space=bass.MemorySpace.PSUM) as pscan:
                for m in range(MT):
                    qht = qpool.tile([128, DSTEP, 128], bf16, tag="qht")
                    nc.sync.dma_start(qht[:], qh_d[m])
                    qet = qepool.tile([128, DE], f32, tag="qet")
                    nc.sync.dma_start(qet[:], qext_d[m])

                    accs = [pscan.tile([128, SUBW], f32, tag=f"acc{s}",
                                       name=f"acc{s}")
                            for s in range(NSUB)]
                    for d in range(DSTEP):
                        for s in range(NSUB):
                            nc.tensor.matmul(accs[s][:], qht[:, d, :],
                                             kht[:, s, d, :],
                                             start=(d == 0),
                                             stop=(d == DSTEP - 1))

                    cv = mpool.tile([128, NSUB * 3], f32, tag="cv")
                    ci = mpool.tile([128, NSUB * 3], f32, tag="ci")
                    for s in range(NSUB):
                        sc = spool.tile([128, SUBW], f32, tag="sc")
                        nc.vector.scalar_tensor_tensor(
                            sc[:], accs[s][:], 1.0,
                            tbias[:, s * SUBW:(s + 1) * SUBW],
                            op0=AL.mult, op1=AL.add)
                        mxv = mpool.tile([128, 8], f32, tag="mxv")
                        mxi = mpool.tile([128, 8], u32, tag="mxi")
                        nc.vector.max_with_indices(mxv[:], mxi[:], sc[:])
                        nc.vector.tensor_copy(cv[:, s * 3:(s + 1) * 3],
                                              mxv[:, 0:3])
                        cif = mpool.tile([128, 3], f32, tag="cif")
                        nc.vector.tensor_copy(cif[:], mxi[:, 0:3])
                        nc.vector.tensor_scalar_add(ci[:, s * 3:(s + 1) * 3],
                                                    cif[:], tbases[:, s:s + 1])

                    cim = mpool.tile([128, NSUB * 3], f32, tag="cim")
                    nc.vector.tensor_scalar_add(cim[:], ci[:], -1.0e6)

                    ers, irs = [], []
                    for r in range(NCAND):
                        vr = mpool.tile([128, 1], f32, tag="vr")
                        nc.vector.tensor_reduce(vr[:], cv[:],
                                                mybir.AxisListType.X, AL.max)
                        mr = mpool.tile([128, NSUB * 3], f32, tag="mr")
                        nc.vector.tensor_scalar(mr[:], cv[:], vr[:, 0:1],
                                                None, op0=AL.is_ge)
                        tt = mpool.tile([128, NSUB * 3], f32, tag="tt")
                        nc.vector.tensor_tensor(tt[:], mr[:], cim[:],
                                                op=AL.mult)
                        tt2 = mpool.tile([128, NSUB * 3], f32, tag="tt2")
                        nc.vector.tensor_scalar_add(tt2[:], tt[:], 1.0e6)
                        ir = mpool.tile([128, 1], f32, tag=f"ir{r}")
                        nc.vector.tensor_reduce(ir[:], tt2[:],
                                                mybir.AxisListType.X, AL.min)
                        if r < NCAND - 1:
                            eq = mpool.tile([128, NSUB * 3], f32, tag="eq")
                            nc.vector.tensor_scalar(eq[:], ci[:], ir[:, 0:1],
                                                    None, op0=AL.is_equal)
                            eqn = mpool.tile([128, NSUB * 3], f32, tag="eqn")
                            nc.vector.tensor_scalar_mul(eqn[:], eq[:], -2.0e9)
                            cv2 = mpool.tile([128, NSUB * 3], f32,
                                             tag=f"cv{r}")
                            nc.vector.tensor_tensor(cv2[:], cv[:], eqn[:],
                                                    op=AL.add)
                            cv = cv2

                        iri = mpool.tile([128, 1], i32, tag="iri")
                        nc.vector.tensor_copy(iri[:], ir[:])
                        g = gpool.tile([128, DE], f32, tag="g")
                        nc.gpsimd.indirect_dma_start(
                            out=g[:], out_offset=None,
                            in_=kext_d[:, :],
                            in_offset=IndirectOffsetOnAxis(ap=iri[:, 0:1],
                                                           axis=0))
                        nc.vector.tensor_tensor(g[:], qet[:], g[:],
                                                op=AL.mult)
                        er = mpool.tile([128, 1], f32, tag=f"er{r}")
                        nc.vector.tensor_reduce(er[:], g[:],
                                                mybir.AxisListType.X, AL.add)
                        ers.append(er)
                        irs.append(ir)

                    b = mpool.tile([128, 1], f32, tag="b")
                    bi = mpool.tile([128, 1], f32, tag="bi")
                    nc.vector.tensor_copy(b[:], ers[0][:])
                    nc.vector.tensor_copy(bi[:], irs[0][:])
                    for r in range(1, NCAND):
                        gt = mpool.tile([128, 1], u32, tag="gt")
                        nc.vector.scalar_tensor_tensor(
                            gt[:], ers[r][:], 1.0, b[:],
                            op0=AL.mult, op1=AL.is_gt)
                        nc.vector.copy_predicated(b[:], gt[:], ers[r][:])
                        nc.vector.copy_predicated(bi[:], gt[:], irs[r][:])
                    nc.vector.tensor_copy(best[:, m:m + 1], b[:])
                    nc.vector.tensor_scalar_add(bix[:, m:m + 1], bi[:],
                                                tcbase[:, 0:1])

            # =================== global argmin via AllReduce(max)+(min)
            cc1 = dpool.tile([128, MT], f32)
            cc2 = dpool.tile([128, MT], f32)
            nc.gpsimd.dma_start(cc1[:], best[:])
            nc.gpsimd.collective_compute(
                "AllReduce", AL.max,
                replica_groups=[list(range(NCORES))],
                ins=[cc1.opt()], outs=[cc2.opt()])
            gbest = wpool.tile([128, MT], f32)
            nc.gpsimd.dma_start(gbest[:], cc2[:])

            ge = wpool.tile([128, MT], f32)
            nc.vector.scalar_tensor_tensor(
                ge[:], best[:], 1.0, gbest[:],
                op0=AL.mult, op1=AL.is_ge)
            t1 = wpool.tile([128, MT], f32)
            nc.vector.tensor_scalar_add(t1[:], bix[:], -1.0e6)
            t2 = wpool.tile([128, MT], f32)
            nc.vector.scalar_tensor_tensor(
                t2[:], ge[:], 1.0, t1[:],
                op0=AL.mult, op1=AL.mult)
            cand = wpool.tile([128, MT], f32)
            nc.vector.tensor_scalar_add(cand[:], t2[:], 1.0e6)

            cc3 = dpool.tile([128, MT], f32)
            cc4 = dpool.tile([128, MT], f32)
            nc.gpsimd.dma_start(cc3[:], cand[:])
            nc.gpsimd.collective_compute(
                "AllReduce", AL.min,
                replica_groups=[list(range(NCORES))],
                ins=[cc3.opt()], outs=[cc4.opt()])
            gif = wpool.tile([128, MT], f32)
            nc.gpsimd.dma_start(gif[:], cc4[:])
            nc.sync.dma_start(gi_d[:], gif[:])

            # winner ids -> flat DRAM, oh-major then transposed to ow-major
            with tc.tile_pool(name="ptr", bufs=1,
                              space=bass.MemorySpace.PSUM) as ptr:
                tpp = ptr.tile([MT, 128], f32, tag="tp")
                nc.tensor.transpose(tpp[:], gif[:], ident[:])
                tpsb = wpool.tile([MT, 128], f32)
                nc.vector.tensor_copy(tpsb[:], tpp[:])
                nc.sync.dma_start(
                    fwin[0:1, 0:LP].rearrange("o (m p) -> m (p o)", p=128)[:],
                    tpsb[:])
                # 53x53 winner grid [oh, ow] -> transpose -> [ow, oh]
                gsb = wpool.tile([OH, OH], f32)
                nc.sync.dma_start(
                    gsb[:],
                    fwin[0:1, 0:L].rearrange("o (a b) -> a (b o)", b=OH)[:])
                tgp = ptr.tile([OH, OH], f32, tag="tp2")
                nc.tensor.transpose(tgp[:], gsb[:], ident[0:OH, 0:OH])
                tgd = wpool.tile([OH, OH], f32)
                nc.vector.tensor_copy(tgd[:], tgp[:])
            nc.sync.dma_start(
                fwinT[0:1, 0:L].rearrange("o (a b) -> a (b o)", b=OH)[:],
                tgd[:])
            padt = wpool.tile([1, FWT_PAD - L], f32)
            nc.vector.memset(padt[:], float(N_MEM))
            nc.sync.dma_start(fwinT[0:1, L:FWT_PAD], padt[:])

            # =================== fold: gather + banded-matmul overlap-add
            with tc.tile_pool(name="pfold", bufs=1,
                              space=bass.MemorySpace.PSUM) as pfold:

                canvasT = wpool.tile([128, H + 2 * PAD, C], f32)
                nc.vector.memset(canvasT[:], 0.0)

                for gidx in range(NFG):
                    nvalid = min(FG, OH - gidx * FG)
                    pxf = mpool.tile([128, 1], f32, tag="pxf")
                    nc.vector.tensor_scalar_add(pxf[:], tpt2[:],
                                                float(FG * gidx))
                    pix = mpool.tile([128, 1], i32, tag="pix")
                    nc.vector.tensor_copy(pix[:], pxf[:])
                    idr = mpool.tile([128, FG], f32, tag="idr")
                    nc.gpsimd.indirect_dma_start(
                        out=idr[:], out_offset=None,
                        in_=fwinT[:, :],
                        in_offset=IndirectOffsetOnAxis(ap=pix[:, 0:1], axis=1))
                    idf = mpool.tile([128, FG], f32, tag="idf")
                    nc.vector.tensor_scalar(idf[:], idr[:], 2.0, None,
                                            op0=AL.mult)
                    idf2 = mpool.tile([128, FG], f32, tag="idf2")
                    nc.vector.tensor_scalar_add(idf2[:], idf[:],
                                                tpkwo[:, 0:1])
                    idi = mpool.tile([128, FG], i32, tag="idi")
                    nc.vector.tensor_copy(idi[:], idf2[:])

                    gv = kpool.tile([128, FG, FSEG], bf16, tag="big")
                    for i in range(nvalid):
                        nc.gpsimd.indirect_dma_start(
                            out=gv[:, i, :], out_offset=None,
                            in_=vals_d[:, :],
                            in_offset=IndirectOffsetOnAxis(
                                ap=idi[:, i:i + 1], axis=0))
                    if DEBUG and gidx == 0:
                        nc.sync.dma_start(dbg_idx[:], idf2[:])
                        gvf = wpool.tile([128, FSEG], f32)
                        nc.vector.tensor_copy(gvf[:], gv[:, 0, :])
                        nc.sync.dma_start(dbg_gv[:], gvf[:])

                    pts = [pfold.tile([128, C * K], f32, tag=f"pt{i}",
                                      name=f"pt{i}")
                           for i in range(nvalid)]
                    for kwl in range(KWG):
                        for i in range(nvalid):
                            gvr = gv[:, i, :].rearrange("p (a k) -> p a k",
                                                        k=KWG)
                            nc.tensor.matmul(pts[i][:],
                                             tbcol[:, kwl, :],
                                             gvr[:, :, kwl],
                                             start=(kwl == 0),
                                             stop=(kwl == KWG - 1))
                    for i in range(nvalid):
                        oh = gidx * FG + i
                        tsb = spool.tile([128, C * K], f32, tag="tsb")
                        nc.vector.tensor_copy(tsb[:], pts[i][:])
                        for c_ in range(C):
                            nc.vector.tensor_tensor(
                                canvasT[:, oh:oh + K, c_:c_ + 1],
                                canvasT[:, oh:oh + K, c_:c_ + 1],
                                tsb[:, c_ * K:(c_ + 1) * K],
                                op=AL.add)

                if DEBUG:
                    nc.sync.dma_start(dbg_cv[:], canvasT[:])
                    fwr = wpool.tile([128, MT], f32)
                    nc.sync.dma_start(
                        fwr[:],
                        fwin[0:1, 0:LP].rearrange(
                            "o (m p) -> p (m o)", p=128)[:])
                    nc.sync.dma_start(dbg_fw[:], fwr[:])

                # ---- normalize (every core has the full canvas) ----
                cvS = wpool.tile([64, 64, C], f32)
                nc.sync.dma_start(
                    cvS[:], canvasT[PAD:PAD + 64, PAD:PAD + 64, :])
                rmax = wpool.tile([64, 1], f32)
                nc.vector.tensor_reduce(rmax[:], cvS[:],
                                        mybir.AxisListType.XY, AL.max)
                gmax = wpool.tile([64, 1], f32)
                nc.gpsimd.partition_all_reduce(
                    gmax[:], rmax[:], 64, ReduceOp.max)
                rcp = wpool.tile([64, 1], f32)
                nc.vector.reciprocal(rcp[:], gmax[:])
                outn = wpool.tile([64, 64, C], f32)
                nc.vector.tensor_scalar(outn[:], cvS[:], rcp[:, 0:1], None,
                                        op0=AL.mult)
                nc.sync.dma_start(out_d[:], outn[:])

    nc.compile()
    return nc


def _get_nc(mode):
    if mode not in _NC_CACHE:
        _NC_CACHE[mode] = _build(mode)
    return _NC_CACHE[mode]


def _im2col(image):
    img = np.ascontiguousarray(image.transpose(2, 0, 1)).astype(np.float32)
    xp = np.pad(img, ((0, 0), (PAD, PAD), (PAD, PAD)))
    win = np.arange(OH)[:, None] + np.arange(K)[None, :]
    p = xp[:, win[:, None, :, None], win[None, :, None, :]]
    return p.transpose(1, 2, 0, 3, 4).reshape(L, D)


def _to6(x, rows, tiles, width):
    # (rows, D) -> (tiles, 128, DSTEP, width) lhsT/rhs layout
    return np.ascontiguousarray(
        x.T.reshape(DSTEP, 128, tiles, width).transpose(2, 1, 0, 3))


def _prepare_in_maps(image, mem_keys, mem_values, mode):
    q = _im2col(image)
    qpad = np.zeros((LP, D), dtype=np.float32)
    qpad[:L] = q

    shared = {}
    shared["qh"] = _to6(qpad.astype(ml_dtypes.bfloat16), LP, MT, 128)

    qext = np.zeros((LP, DE), dtype=np.float32)
    qext[:, :D] = qpad
    qext[:, D] = 1.0
    shared["qext"] = qext.reshape(MT, 128, DE)

    shared["bases"] = np.ascontiguousarray(np.broadcast_to(
        (np.arange(NSUB, dtype=np.float32) * SUBW), (128, NSUB)))

    # values table [r*2 + kw//16, (c, kh, kw%16)] bf16, + dummy zero rows
    vt = np.zeros((VROWS, FSEG), dtype=ml_dtypes.bfloat16)
    vt[:N_MEM * NKWO] = (
        mem_values.reshape(N_MEM, C, K, NKWO, KWG)
        .transpose(0, 3, 1, 2, 4).reshape(N_MEM * NKWO, FSEG)
        .astype(ml_dtypes.bfloat16))
    shared["vals"] = vt

    # banded fold matrices: bcol[kwl][p, x] = 1 iff x == p//2 + 16*(p%2) + kwl
    p_ar = np.arange(128)
    xtgt = p_ar // 2 + KWG * (p_ar % 2)
    bc = np.zeros((KWG, 128, 128), dtype=ml_dtypes.bfloat16)
    for kwl in range(KWG):
        valid = p_ar < OH * NKWO
        bc[kwl, p_ar[valid], (xtgt + kwl)[valid]] = 1.0
    shared["bcol"] = bc

    # pt2[p]: ow-major winner-array base for ow = p//2 (pad zone if ow >= OH)
    shared["pt2"] = ((p_ar // NKWO) * OH).astype(np.float32).reshape(128, 1)
    shared["pkwo"] = (p_ar % NKWO).astype(np.float32).reshape(128, 1)

    in_maps = []
    for c in range(NCORES):
        kc = mem_keys[c * NK:(c + 1) * NK]
        kcp = np.zeros((NKP, D), dtype=np.float32)
        kcp[:NK] = kc
        m = dict(shared)
        m["kh"] = _to6(kcp.astype(ml_dtypes.bfloat16), NKP, NSUB, SUBW)

        n2 = (kc.astype(np.float64) ** 2).sum(axis=1)
        biasv = np.full(NKP, -1.0e9, dtype=np.float32)
        biasv[:NK] = (-0.5 * n2).astype(np.float32)
        m["bias"] = np.ascontiguousarray(
            np.broadcast_to(biasv, (128, NKP))).astype(np.float32)

        kext = np.zeros((NKP, DE), dtype=np.float32)
        kext[:, :D] = kcp
        kext[:, D] = biasv
        m["kext"] = kext

        m["cbase"] = np.full((128, 1), float(c * NK), dtype=np.float32)
        in_maps.append(m)
    return in_maps


def kernel(**inputs):
    global LAST_EXEC_NS, LAST_RESULTS
    image = np.asarray(inputs["image"], dtype=np.float32)
    mem_keys = np.asarray(inputs["mem_keys"], dtype=np.float32)
    mem_values = np.asarray(inputs["mem_values"], dtype=np.float32)

    mode = MODE
    nc = _get_nc(mode)
    in_maps = _prepare_in_maps(image, mem_keys, mem_values, mode)

    r = run_bass_kernel_spmd(nc, in_maps, list(range(NCORES)), trace=TRACE)
    LAST_EXEC_NS = r.exec_time_ns
    LAST_RESULTS = r.results
    out = r.results[0]["out"]  # [x(W), y(H), c]
    return np.ascontiguousarray(out.transpose(1, 0, 2)).astype(np.float32)


if __name__ == "__main__":
    rng = np.random.default_rng(0)
    ins = dict(
        image=rng.random((H, W, C), dtype=np.float32),
        mem_keys=rng.standard_normal((N_MEM, D), dtype=np.float32),
        mem_values=rng.standard_normal((N_MEM, D), dtype=np.float32),
    )
    o = kernel(**ins)
    print("out", o.shape, o.dtype, float(o.max()), float(o.min()))
